# revision 4
# baseline (speedup 1.0000x reference)
# Banded (sliding-window) multi-head attention for Trainium2, 8 NeuronCores.
# Sharding: batch x head-group (2 batches x 4 groups of 4 heads). QKV is
# column-parallel, W_o row-parallel (megatron TP); host sums the 4 partial
# outputs per batch and adds b_o.
import math
import numpy as np
import ml_dtypes

import concourse.bass as bass
import concourse.tile as tile
import concourse.mybir as mybir
import concourse.bass_utils as bass_utils

F32 = mybir.dt.float32
F32R = mybir.dt.float32r
BF16 = mybir.dt.bfloat16
BF16NP = ml_dtypes.bfloat16

B, S, DIN, EMB, NH, WIN = 2, 4096, 1024, 1024, 16, 256
HD = EMB // NH          # 64
W = WIN // 2            # 128 one-sided window
HL = 4                  # heads per core
NKC = S // 128          # 32 key chunks
NQT = S // 512          # 8 query tiles
NEGB = -1e15            # exp(x + NEGB) == 0 in fp32
SCALE = 1.0 / math.sqrt(HD)

_MAX_WAITS = 1


def _split_excess_waits(nc):
    # This walrus build accepts a single sync-wait per instruction; move
    # excess waits onto same-engine NoOps inserted immediately before.
    for bb in nc.m.functions[0].blocks:
        new_insts = []
        changed = False
        for inst in bb.instructions:
            si = inst.sync_info
            if si is not None and si.on_wait and len(si.on_wait) > _MAX_WAITS:
                waits = list(si.on_wait)
                head, tail = waits[:-_MAX_WAITS], waits[-_MAX_WAITS:]
                k = 0
                while head:
                    chunk, head = head[:_MAX_WAITS], head[_MAX_WAITS:]
                    new_insts.append(mybir.InstNoOp(
                        name=f"{inst.name}-ws{k}", engine=inst.engine,
                        ins=[], outs=[],
                        sync_info=mybir.SyncInfo(on_wait=chunk, on_update=[])))
                    k += 1
                inst.sync_info = mybir.SyncInfo(
                    on_wait=tail, on_update=list(si.on_update))
                changed = True
            new_insts.append(inst)
        if changed:
            bb.instructions = new_insts


def _build_program():
    nc = bass.Bass("TRN2", target_bir_lowering=False, debug=False)
    xT = nc.dram_tensor("xT", [DIN, S], F32R, kind="ExternalInput").ap()
    wqk = nc.dram_tensor("wqk", [DIN, HL * 128], F32R, kind="ExternalInput").ap()
    wv = nc.dram_tensor("wv", [DIN, HL * 64], F32R, kind="ExternalInput").ap()
    wo = nc.dram_tensor("wo", [HL * 64, EMB], F32R, kind="ExternalInput").ap()
    bqk = nc.dram_tensor("bqk", [128, HL], F32, kind="ExternalInput").ap()
    bv = nc.dram_tensor("bv", [1, HL * 64], F32R, kind="ExternalInput").ap()
    padb = nc.dram_tensor("padb", [128, NKC], F32, kind="ExternalInput").ap()
    mask = nc.dram_tensor("mask", [128, 384], BF16, kind="ExternalInput").ap()
    onesr = nc.dram_tensor("onesr", [1, 128], F32R, kind="ExternalInput").ap()
    y = nc.dram_tensor("y", [S, EMB], F32, kind="ExternalOutput").ap()

    import contextlib
    with tile.TileContext(nc) as tc, contextlib.ExitStack() as ctx:
        cpool = ctx.enter_context(tc.tile_pool(name="const", bufs=1))
        xpool = ctx.enter_context(tc.tile_pool(name="x", bufs=16))
        qkpool = ctx.enter_context(tc.tile_pool(name="qkt", bufs=1))
        vpool = ctx.enter_context(tc.tile_pool(name="v", bufs=1))
        ptpool = ctx.enter_context(tc.tile_pool(name="pt", bufs=8))
        opool = ctx.enter_context(tc.tile_pool(name="outp", bufs=1))
        rpool = ctx.enter_context(tc.tile_pool(name="rec", bufs=2))
        ypool = ctx.enter_context(tc.tile_pool(name="y", bufs=3))
        pp = ctx.enter_context(tc.tile_pool(name="pp", bufs=2, space="PSUM"))
        stp = ctx.enter_context(tc.tile_pool(name="st", bufs=2, space="PSUM"))
        pvp = ctx.enter_context(tc.tile_pool(name="pv", bufs=2, space="PSUM"))
        bcp = ctx.enter_context(tc.tile_pool(name="bc", bufs=2, space="PSUM"))

        # constants
        wqk_t = []
        for kk in range(8):
            t = cpool.tile([128, HL * 128], F32R, tag=f"wqk{kk}")
            nc.sync.dma_start(t[:], wqk[kk * 128:(kk + 1) * 128, :])
            wqk_t.append(t)
        wv_t = []
        for kk in range(8):
            t = cpool.tile([128, HL * 64], F32R, tag=f"wv{kk}")
            nc.sync.dma_start(t[:], wv[kk * 128:(kk + 1) * 128, :])
            wv_t.append(t)
        wo_t = []
        for p in range(2):
            t = cpool.tile([128, EMB], F32R, tag=f"wo{p}")
            nc.sync.dma_start(t[:], wo[p * 128:(p + 1) * 128, :])
            wo_t.append(t)
        bqk_t = cpool.tile([128, HL], F32, tag="bqk")
        nc.sync.dma_start(bqk_t[:], bqk[:])
        bv_t = cpool.tile([1, HL * 64], F32R, tag="bv")
        nc.sync.dma_start(bv_t[:], bv[:])
        padb_t = cpool.tile([128, NKC], F32, tag="padb")
        nc.sync.dma_start(padb_t[:], padb[:])
        mask_t = cpool.tile([128, 384], BF16, tag="mask")
        nc.sync.dma_start(mask_t[:], mask[:])
        onesr_t = cpool.tile([1, 128], F32R, tag="onesr")
        nc.sync.dma_start(onesr_t[:], onesr[:])
        onesb_t = cpool.tile([1, 65], BF16, tag="onesb")
        nc.vector.memset(onesb_t[:], 1.0)
        zrow_t = cpool.tile([1, 512], BF16, tag="zrow")
        nc.vector.memset(zrow_t[:], 0.0)

        # phase 1: QKV projection.
        # qT2[p] = q^T for heads 2p (partitions 0:64) and 2p+1 (64:128), bf16
        # kT2[p] = k^T likewise; v[kc] = [128 tok, 4*65] bf16 (+ones col)
        qT2 = [qkpool.tile([128, S], BF16, tag=f"qt{p}", name=f"qt{p}")
               for p in range(2)]
        kT2 = [qkpool.tile([128, S], BF16, tag=f"kt{p}", name=f"kt{p}")
               for p in range(2)]
        v_t = [vpool.tile([128, HL * 65], BF16, tag=f"v{kc}", name=f"v{kc}")
               for kc in range(NKC)]
        for tt in range(8):
            xs = []
            for kk in range(8):
                t = xpool.tile([128, 512], F32R, tag="xt")
                nc.sync.dma_start(t[:], xT[kk * 128:(kk + 1) * 128,
                                            tt * 512:(tt + 1) * 512])
                xs.append(t)
            for j in range(HL):
                ps = pp.tile([128, 512], F32, tag="pp")
                for kk in range(8):
                    nc.tensor.matmul(ps[:], wqk_t[kk][:, j * 128:(j + 1) * 128],
                                     xs[kk][:], start=(kk == 0), stop=(kk == 7))
                p2, hf = j // 2, (j % 2) * 64
                nc.vector.tensor_scalar_add(
                    qT2[p2][hf:hf + 64, tt * 512:(tt + 1) * 512],
                    ps[0:64, :], bqk_t[0:64, j:j + 1])
                nc.vector.tensor_scalar_add(
                    kT2[p2][hf:hf + 64, tt * 512:(tt + 1) * 512],
                    ps[64:128, :], bqk_t[64:128, j:j + 1])
            for m in range(4):
                ps = pp.tile([128, HL * 64], F32, tag="pp")
                for kk in range(8):
                    nc.tensor.matmul(ps[:], xs[kk][:, m * 128:(m + 1) * 128],
                                     wv_t[kk][:], start=(kk == 0), stop=False)
                nc.tensor.matmul(ps[:], onesr_t[:], bv_t[:],
                                 start=False, stop=True)
                kc = tt * 4 + m
                vv = v_t[kc][:].rearrange("p (h c) -> p h c", h=HL)
                nc.vector.tensor_copy(
                    vv[:, :, 0:64], ps[:].rearrange("p (h c) -> p h c", h=HL))
                nc.vector.memset(vv[:, :, 64:65], 1.0)

        # phase 2: banded attention per head
        def pv_group(j, qt, pts):
            t0 = qt * 512
            ps = pvp.tile([65, 512], F32, tag="pv")
            nc.tensor.matmul(ps[:], onesb_t[:], zrow_t[:], start=True, stop=False)
            ks = [kc for kc in range(4 * qt - 1, 4 * qt + 5) if 0 <= kc < NKC]
            for i, kc in enumerate(ks):
                qlo = max(0, 128 * kc - 128)
                qhi = min(S, 128 * kc + 256)
                a = max(qlo, t0)
                b2 = min(qhi, t0 + 512)
                nc.tensor.matmul(
                    ps[:, a - t0:b2 - t0],
                    v_t[kc][:, j * 65:(j + 1) * 65],
                    pts[kc][:, a - qlo:b2 - qlo],
                    start=False, stop=(i == len(ks) - 1))
            rec = rpool.tile([1, 512], F32R, tag="rec")
            with nc.allow_low_precision(reason="f32r rounding for matmul"):
                nc.vector.reciprocal(rec[:], ps[64:65, :])
            bc = bcp.tile([64, 512], F32, tag="bc")
            nc.tensor.matmul(bc[:], onesr_t[:, 0:64], rec[:], start=True, stop=True)
            rec64 = rpool.tile([64, 512], F32, tag="rec64")
            nc.vector.tensor_copy(rec64[:], bc[:])
            p, half = j // 2, j % 2
            with nc.allow_low_precision(reason="f32r rounding for matmul"):
                nc.vector.tensor_mul(
                    outp[p][qt][half * 64:half * 64 + 64, :], ps[0:64, :], rec64[:])

        outp = [[opool.tile([128, 512], F32R, tag=f"op{p}_{qt}", name=f"op{p}_{qt}")
                 for qt in range(NQT)] for p in range(2)]
        for j in range(HL):
            pts = {}
            for kc in range(NKC):
                qlo = max(0, 128 * kc - 128)
                qhi = min(S, 128 * kc + 256)
                w = qhi - qlo
                st = stp.tile([128, w], F32, tag="st")
                p2, hf = j // 2, (j % 2) * 64
                nc.tensor.matmul(st[:],
                                 kT2[p2][hf:hf + 64, kc * 128:(kc + 1) * 128],
                                 qT2[p2][hf:hf + 64, qlo:qhi],
                                 start=True, stop=True)
                pt = ptpool.tile([128, w], BF16, tag="pt")
                nc.scalar.activation(pt[:], st[:], mybir.ActivationFunctionType.Exp,
                                     bias=padb_t[:, kc:kc + 1], scale=SCALE)
                ms = qlo - 128 * kc + 128
                nc.vector.tensor_mul(pt[:], pt[:], mask_t[:, ms:ms + w])
                pts[kc] = pt
                if kc % 4 == 0 and kc >= 4:
                    pv_group(j, kc // 4 - 1, pts)
            pv_group(j, NQT - 1, pts)

        # phase 3: output projection (partial y; host adds b_o and reduces)
        for qt in range(NQT):
            for m in range(4):
                for n in range(2):
                    ps = pp.tile([128, 512], F32, tag="pp")
                    for p in range(2):
                        nc.tensor.matmul(ps[:], outp[p][qt][:, m * 128:(m + 1) * 128],
                                         wo_t[p][:, n * 512:(n + 1) * 512],
                                         start=(p == 0), stop=(p == 1))
                    yt = ypool.tile([128, 512], F32, tag="y")
                    nc.vector.tensor_copy(yt[:], ps[:])
                    nc.sync.dma_start(
                        y[qt * 512 + m * 128:qt * 512 + (m + 1) * 128,
                          n * 512:(n + 1) * 512], yt[:])

    _split_excess_waits(nc)
    return nc


_NC = None


def _get_nc():
    global _NC
    if _NC is None:
        _NC = _build_program()
    return _NC


def kernel(x, padding_mask, W_qkv, b_qkv, W_o, b_o):
    x = np.asarray(x, np.float32)
    padding_mask = np.asarray(padding_mask)
    W_qkv = np.asarray(W_qkv, np.float32)
    b_qkv = np.asarray(b_qkv, np.float32)
    W_o = np.asarray(W_o, np.float32)
    b_o = np.asarray(b_o, np.float32)

    # band mask (transposed layout): key j_local (partition), query col i
    # covering queries [128*kc - 128, 128*kc + 256); valid iff j <= i <= j+256
    jj = np.arange(128)[:, None]
    ii = np.arange(384)[None, :]
    mask01 = ((ii >= jj) & (ii <= jj + 2 * W)).astype(BF16NP)
    ones_r = np.ones((1, 128), np.float32)

    in_maps = []
    for c in range(8):
        b = c // 4
        g = c % 4
        heads = [g * HL + j for j in range(HL)]
        xT = np.ascontiguousarray(x[b].T)
        wqk = np.empty((DIN, HL * 128), np.float32)
        wv = np.empty((DIN, HL * 64), np.float32)
        wo = np.empty((HL * 64, EMB), np.float32)
        bqk = np.empty((128, HL), np.float32)
        bv = np.empty((1, HL * 64), np.float32)
        for j, h in enumerate(heads):
            wqk[:, j * 128:(j + 1) * 128] = W_qkv[:, h * 192:h * 192 + 128]
            wv[:, j * 64:(j + 1) * 64] = W_qkv[:, h * 192 + 128:h * 192 + 192]
            wo[j * 64:(j + 1) * 64, :] = W_o[h * 64:(h + 1) * 64, :]
            bqk[:, j] = b_qkv[h * 192:h * 192 + 128]
            bv[0, j * 64:(j + 1) * 64] = b_qkv[h * 192 + 128:h * 192 + 192]
        padb = np.where(padding_mask[b].reshape(NKC, 128).T.astype(bool),
                        0.0, NEGB).astype(np.float32)
        in_maps.append({
            "xT": xT, "wqk": wqk, "wv": wv, "wo": wo, "bqk": bqk, "bv": bv,
            "padb": np.ascontiguousarray(padb), "mask": mask01, "onesr": ones_r,
        })

    nc = _get_nc()
    res = bass_utils.run_bass_kernel_spmd(nc, in_maps, core_ids=list(range(8)))
    out = np.zeros((B, S, EMB), np.float32)
    for c in range(8):
        out[c // 4] += res.results[c]["y"]
    out += b_o
    return out


# revision 17
# speedup vs baseline: 1.3797x; 1.3797x over previous
# Banded (sliding-window) multi-head attention for Trainium2, 8 NeuronCores.
# Sharding: batch x head-group (2 batches x 4 groups of 4 heads). QKV is
# column-parallel, W_o row-parallel (megatron TP); host sums the 4 partial
# outputs per batch and adds b_o.
import math
import numpy as np
import ml_dtypes

import concourse.bass as bass
import concourse.tile as tile
import concourse.mybir as mybir
import concourse.bass_utils as bass_utils

F32 = mybir.dt.float32
F32R = mybir.dt.float32r
BF16 = mybir.dt.bfloat16
BF16NP = ml_dtypes.bfloat16

B, S, DIN, EMB, NH, WIN = 2, 4096, 1024, 1024, 16, 256
HD = EMB // NH          # 64
W = WIN // 2            # 128 one-sided window
HL = 4                  # heads per core
NKC = S // 128          # 32 key chunks
NQT = S // 512          # 8 query tiles
NEGB = -1e15            # exp(x + NEGB) == 0 in fp32
SCALE = 1.0 / math.sqrt(HD)

_MAX_WAITS = 1


def _split_excess_waits(nc):
    # This walrus build accepts a single sync-wait per instruction; move
    # excess waits onto same-engine NoOps inserted immediately before.
    for bb in nc.m.functions[0].blocks:
        new_insts = []
        changed = False
        for inst in bb.instructions:
            si = inst.sync_info
            if si is not None and si.on_wait and len(si.on_wait) > _MAX_WAITS:
                waits = list(si.on_wait)
                head, tail = waits[:-_MAX_WAITS], waits[-_MAX_WAITS:]
                k = 0
                while head:
                    chunk, head = head[:_MAX_WAITS], head[_MAX_WAITS:]
                    new_insts.append(mybir.InstNoOp(
                        name=f"{inst.name}-ws{k}", engine=inst.engine,
                        ins=[], outs=[],
                        sync_info=mybir.SyncInfo(on_wait=chunk, on_update=[])))
                    k += 1
                inst.sync_info = mybir.SyncInfo(
                    on_wait=tail, on_update=list(si.on_update))
                changed = True
            new_insts.append(inst)
        if changed:
            bb.instructions = new_insts


def _build_program(with_bias):
    nc = bass.Bass("TRN2", target_bir_lowering=False, debug=False)
    xT = nc.dram_tensor("xT", [DIN, S], F32R, kind="ExternalInput").ap()
    wqk = nc.dram_tensor("wqk", [DIN, HL * 128], F32R, kind="ExternalInput").ap()
    wv = nc.dram_tensor("wv", [DIN, HL * 64], F32R, kind="ExternalInput").ap()
    wo = nc.dram_tensor("wo", [HL * 64, EMB], F32R, kind="ExternalInput").ap()
    bqkT = nc.dram_tensor("bqkT", [1, HL * 128], F32R, kind="ExternalInput").ap()
    bv = nc.dram_tensor("bv", [1, HL * 64], F32R, kind="ExternalInput").ap()
    padb = nc.dram_tensor("padb", [128, NKC], F32, kind="ExternalInput").ap()
    mask = nc.dram_tensor("mask", [128, 384], BF16, kind="ExternalInput").ap()
    onesr = nc.dram_tensor("onesr", [1, 512], F32R, kind="ExternalInput").ap()
    y = nc.dram_tensor("y", [S, EMB], F32, kind="ExternalOutput").ap()

    import contextlib
    with tile.TileContext(nc) as tc, contextlib.ExitStack() as ctx:
        cpool = ctx.enter_context(tc.tile_pool(name="const", bufs=1))
        xpool = ctx.enter_context(tc.tile_pool(name="x", bufs=16))
        qkpool = ctx.enter_context(tc.tile_pool(name="qkt", bufs=1))
        vpool = ctx.enter_context(tc.tile_pool(name="v", bufs=1))
        ptpool = ctx.enter_context(tc.tile_pool(name="pt", bufs=40))
        opool = ctx.enter_context(tc.tile_pool(name="outp", bufs=4))
        rpool = ctx.enter_context(tc.tile_pool(name="rec", bufs=2))
        ypool = ctx.enter_context(tc.tile_pool(name="y", bufs=4))
        pp = ctx.enter_context(tc.tile_pool(name="pp", bufs=2, space="PSUM"))
        ypp = ctx.enter_context(tc.tile_pool(name="ypp", bufs=2, space="PSUM"))
        stp = ctx.enter_context(tc.tile_pool(name="st", bufs=2, space="PSUM"))
        pvp = ctx.enter_context(tc.tile_pool(name="pv", bufs=2, space="PSUM"))

        # constants
        wqk_t = []
        for kk in range(8):
            t = cpool.tile([128, HL * 128], F32R, tag=f"wqk{kk}", name=f"wqk{kk}")
            nc.sync.dma_start(t[:], wqk[kk * 128:(kk + 1) * 128, :])
            wqk_t.append(t)

        # phase 1: QKV projection.
        # qT2[p] = q^T for heads 2p (partitions 0:64) and 2p+1 (64:128), bf16
        # kT2[p] = k^T likewise; v[kc] = [128 tok, 4*65] bf16 (+ones col)
        qT2 = [qkpool.tile([128, S], BF16, tag=f"qt{p}", name=f"qt{p}")
               for p in range(2)]
        kT2 = [qkpool.tile([128, S], BF16, tag=f"kt{p}", name=f"kt{p}")
               for p in range(2)]
        v_t = [vpool.tile([128, HL * 65], BF16, tag=f"v{kc}", name=f"v{kc}")
               for kc in range(NKC)]
        def qkv_tile(tt):
            if tt == 0:
                xs = x0
            else:
                xs = []
                for kk in range(8):
                    t = xpool.tile([128, 512], F32R, tag="xt", name=f"xt{tt}_{kk}")
                    nc.sync.dma_start(t[:], xT[kk * 128:(kk + 1) * 128,
                                                tt * 512:(tt + 1) * 512])
                    xs.append(t)
            for qk in range(2):          # 0 -> q of all heads, 1 -> k
                for p2 in range(2):      # head pair
                    c0 = qk * 256 + p2 * 128
                    ps = pp.tile([128, 512], F32, tag="pp", name=f"qk{tt}_{qk}_{p2}")
                    for kk in range(8):
                        nc.tensor.matmul(ps[:], wqk_t[kk][:, c0:c0 + 128],
                                         xs[kk][:], start=(kk == 0),
                                         stop=(kk == 7 and not with_bias))
                    if with_bias:
                        nc.tensor.matmul(ps[:], bqkT_t[:, c0:c0 + 128],
                                         onesr_t[:], start=False, stop=True)
                    dst = qT2[p2] if qk == 0 else kT2[p2]
                    nc.scalar.copy(dst[:, tt * 512:(tt + 1) * 512], ps[:])
            for m in range(4):
                ps = pp.tile([128, HL * 64], F32, tag="pp", name=f"v{tt}_{m}")
                for kk in range(8):
                    nc.tensor.matmul(ps[:], xs[kk][:, m * 128:(m + 1) * 128],
                                     wv_t[kk][:], start=(kk == 0),
                                     stop=(kk == 7 and not with_bias))
                if with_bias:
                    nc.tensor.matmul(ps[:], onesr_t[:, 0:128], bv_t[:],
                                     start=False, stop=True)
                kc = tt * 4 + m
                vv = v_t[kc][:].rearrange("p (h c) -> p h c", h=HL)
                nc.vector.tensor_copy(
                    vv[:, :, 0:64], ps[:].rearrange("p (h c) -> p h c", h=HL))
                nc.gpsimd.memset(vv[:, :, 64:65], 1.0)

        wo_t = []
        for p in range(2):
            t = cpool.tile([128, EMB], F32R, tag=f"wo{p}", name=f"wo{p}")
            nc.sync.dma_start(t[:], wo[p * 128:(p + 1) * 128, :])
            wo_t.append(t)

        # phase 2+3 interleaved per 512-query tile:
        #   pair-packed P^T production (QK -> exp -> band mask on GPSIMD),
        #   PV accumulate out^T, normalize, O-projection + y DMA.
        pts = [{} for _ in range(HL)]

        def make_pt(j, kc):
            qlo = max(0, 128 * kc - 128)
            qhi = min(S, 128 * kc + 256)
            w = qhi - qlo
            p2, hf = j // 2, (j % 2) * 64
            st = stp.tile([128, w], F32, tag="st", name=f"st{j}_{kc}")
            nc.tensor.matmul(st[:],
                             kT2[p2][hf:hf + 64, kc * 128:(kc + 1) * 128],
                             qT2[p2][hf:hf + 64, qlo:qhi],
                             start=True, stop=True)
            pt = ptpool.tile([128, w], BF16, tag="pt", name=f"pt{j}_{kc}")
            nc.scalar.activation(pt[:], st[:], mybir.ActivationFunctionType.Exp,
                                 bias=padb_t[:, kc:kc + 1], scale=SCALE)
            # band mask via affine_select on the edge regions (middle third of
            # the generic 384-wide window is always valid).
            ms = qlo - 128 * kc + 128
            lo_w = max(0, 128 - ms)
            hi_s = max(0, 256 - ms)
            if lo_w > 0:
                # keep iff i - j >= 0 with i = ms + y; expr = ms + y - x
                nc.gpsimd.affine_select(
                    out=pt[:, 0:lo_w], in_=pt[:, 0:lo_w],
                    compare_op=mybir.AluOpType.is_ge, fill=0.0,
                    base=ms, pattern=[[1, lo_w]], channel_multiplier=-1)
            if hi_s < w:
                # keep iff j + 256 - i >= 0 with i = ms + hi_s + y
                nc.gpsimd.affine_select(
                    out=pt[:, hi_s:w], in_=pt[:, hi_s:w],
                    compare_op=mybir.AluOpType.is_ge, fill=0.0,
                    base=256 - ms - hi_s, pattern=[[-1, w - hi_s]],
                    channel_multiplier=1)
            pts[j][kc] = (pt, 0)

        def pv_group(j, qt, outp):
            t0 = qt * 512
            ps = pvp.tile([65, 512], F32, tag="pv", name=f"pv{j}_{qt}")
            nc.vector.memset(ps[:], 0.0)
            ks = [kc for kc in range(4 * qt - 1, 4 * qt + 5) if 0 <= kc < NKC]
            for i, kc in enumerate(ks):
                qlo = max(0, 128 * kc - 128)
                qhi = min(S, 128 * kc + 256)
                a = max(qlo, t0)
                b2 = min(qhi, t0 + 512)
                pt, off = pts[j][kc]
                nc.tensor.matmul(
                    ps[:, a - t0:b2 - t0],
                    v_t[kc][:, j * 65:(j + 1) * 65],
                    pt[:, off + a - qlo:off + b2 - qlo],
                    start=False, stop=(i == len(ks) - 1),
                    skip_group_check=True)
            rec = rpool.tile([1, 512], F32R, tag="rec", name=f"rec{j}_{qt}")
            with nc.allow_low_precision(reason="f32r rounding for matmul"):
                nc.vector.reciprocal(rec[:], ps[64:65, :])
            bc = ypp.tile([64, 512], F32, tag="ypp", name=f"bc{j}_{qt}")
            nc.tensor.matmul(bc[:], onesr_t[:, 0:64], rec[:], start=True, stop=True)
            rec64 = rpool.tile([64, 512], F32, tag="rec64", name=f"r64{j}_{qt}")
            nc.scalar.copy(rec64[:], bc[:])
            p, half = j // 2, j % 2
            with nc.allow_low_precision(reason="f32r rounding for matmul"):
                nc.vector.tensor_mul(
                    outp[p][half * 64:half * 64 + 64, :], ps[0:64, :], rec64[:])

        def attn_tile(qt):
            lo = 4 * qt + 1 if qt > 0 else 0
            hi = min(4 * qt + 5, NKC)
            for kc in range(lo, hi):
                for j in range(HL):
                    make_pt(j, kc)
            outp = [opool.tile([128, 512], F32R, tag=f"op{p}", name=f"op{p}_{qt}")
                    for p in range(2)]
            for j in range(HL):
                pv_group(j, qt, outp)
            for m in range(4):
                for n in range(2):
                    ps = ypp.tile([128, 512], F32, tag="ypp", name=f"y{qt}_{m}_{n}")
                    for p in range(2):
                        nc.tensor.matmul(ps[:], outp[p][:, m * 128:(m + 1) * 128],
                                         wo_t[p][:, n * 512:(n + 1) * 512],
                                         start=(p == 0), stop=(p == 1))
                    yt = ypool.tile([128, 512], F32, tag="y",
                                    name=f"yt{qt}_{m}_{n}")
                    if (m + n) % 2 == 0:
                        nc.vector.tensor_copy(yt[:], ps[:])
                    else:
                        nc.scalar.copy(yt[:], ps[:])
                    nc.sync.dma_start(
                        y[qt * 512 + m * 128:qt * 512 + (m + 1) * 128,
                          n * 512:(n + 1) * 512], yt[:])

        x0 = []
        for kk in range(8):
            t = xpool.tile([128, 512], F32R, tag="xt", name=f"xt0_{kk}")
            nc.sync.dma_start(t[:], xT[kk * 128:(kk + 1) * 128, 0:512])
            x0.append(t)
        wv_t = []
        for kk in range(8):
            t = cpool.tile([128, HL * 64], F32R, tag=f"wv{kk}", name=f"wv{kk}")
            nc.sync.dma_start(t[:], wv[kk * 128:(kk + 1) * 128, :])
            wv_t.append(t)
        bqkT_t = cpool.tile([1, HL * 128], F32R, tag="bqkT")
        nc.sync.dma_start(bqkT_t[:], bqkT[:])
        bv_t = cpool.tile([1, HL * 64], F32R, tag="bv")
        nc.sync.dma_start(bv_t[:], bv[:])
        padb_t = cpool.tile([128, NKC], F32, tag="padb")
        nc.sync.dma_start(padb_t[:], padb[:])
        mask_t = cpool.tile([128, 384], BF16, tag="mask")
        nc.sync.dma_start(mask_t[:], mask[:])
        onesr_t = cpool.tile([1, 512], F32R, tag="onesr")
        nc.sync.dma_start(onesr_t[:], onesr[:])
        onesb_t = cpool.tile([1, 65], BF16, tag="onesb")
        nc.vector.memset(onesb_t[:], 1.0)
        zrow_t = cpool.tile([1, 512], BF16, tag="zrow")
        nc.vector.memset(zrow_t[:], 0.0)
        for tt in range(8):
            qkv_tile(tt)
            if tt >= 1:
                attn_tile(tt - 1)
        attn_tile(NQT - 1)

    _split_excess_waits(nc)
    return nc


_NC = {}


def _get_nc(with_bias=False):
    if with_bias not in _NC:
        _NC[with_bias] = _build_program(with_bias)
    return _NC[with_bias]


def kernel(x, padding_mask, W_qkv, b_qkv, W_o, b_o):
    x = np.asarray(x, np.float32)
    padding_mask = np.asarray(padding_mask)
    W_qkv = np.asarray(W_qkv, np.float32)
    b_qkv = np.asarray(b_qkv, np.float32)
    W_o = np.asarray(W_o, np.float32)
    b_o = np.asarray(b_o, np.float32)

    # band mask (transposed layout): key j_local (partition), query col i
    # covering queries [128*kc - 128, 128*kc + 256); valid iff j <= i <= j+256
    jj = np.arange(128)[:, None]
    ii = np.arange(384)[None, :]
    mask01 = ((ii >= jj) & (ii <= jj + 2 * W)).astype(BF16NP)
    ones_r = np.ones((1, 512), np.float32)

    in_maps = []
    for c in range(8):
        b = c // 4
        g = c % 4
        heads = [g * HL + j for j in range(HL)]
        xT = np.ascontiguousarray(x[b].T)
        wqk = np.empty((DIN, HL * 128), np.float32)
        wv = np.empty((DIN, HL * 64), np.float32)
        wo = np.empty((HL * 64, EMB), np.float32)
        bqkT = np.empty((1, HL * 128), np.float32)
        bv = np.empty((1, HL * 64), np.float32)
        for j, h in enumerate(heads):
            wqk[:, j * 64:(j + 1) * 64] = W_qkv[:, h * 192:h * 192 + 64]
            wqk[:, 256 + j * 64:256 + (j + 1) * 64] = \
                W_qkv[:, h * 192 + 64:h * 192 + 128]
            wv[:, j * 64:(j + 1) * 64] = W_qkv[:, h * 192 + 128:h * 192 + 192]
            wo[j * 64:(j + 1) * 64, :] = W_o[h * 64:(h + 1) * 64, :]
            bqkT[0, j * 64:(j + 1) * 64] = b_qkv[h * 192:h * 192 + 64]
            bqkT[0, 256 + j * 64:256 + (j + 1) * 64] = \
                b_qkv[h * 192 + 64:h * 192 + 128]
            bv[0, j * 64:(j + 1) * 64] = b_qkv[h * 192 + 128:h * 192 + 192]
        padb = np.where(padding_mask[b].reshape(NKC, 128).T.astype(bool),
                        0.0, NEGB).astype(np.float32)
        in_maps.append({
            "xT": xT, "wqk": wqk, "wv": wv, "wo": wo, "bqkT": bqkT, "bv": bv,
            "padb": np.ascontiguousarray(padb), "mask": mask01, "onesr": ones_r,
        })

    with_bias = bool(np.any(b_qkv != 0))
    nc = _get_nc(with_bias)
    res = bass_utils.run_bass_kernel_spmd(nc, in_maps, core_ids=list(range(8)))
    out = np.zeros((B, S, EMB), np.float32)
    for c in range(8):
        out[c // 4] += res.results[c]["y"]
    out += b_o
    return out


# revision 19
# speedup vs baseline: 1.4050x; 1.0184x over previous
# Banded (sliding-window) multi-head attention for Trainium2, 8 NeuronCores.
# Sharding: batch x head-group (2 batches x 4 groups of 4 heads). QKV is
# column-parallel, W_o row-parallel (megatron TP); host sums the 4 partial
# outputs per batch and adds b_o.
import math
import numpy as np
import ml_dtypes

import concourse.bass as bass
import concourse.tile as tile
import concourse.mybir as mybir
import concourse.bass_utils as bass_utils

F32 = mybir.dt.float32
F32R = mybir.dt.float32r
BF16 = mybir.dt.bfloat16
BF16NP = ml_dtypes.bfloat16

B, S, DIN, EMB, NH, WIN = 2, 4096, 1024, 1024, 16, 256
HD = EMB // NH          # 64
W = WIN // 2            # 128 one-sided window
HL = 4                  # heads per core
NKC = S // 128          # 32 key chunks
NQT = S // 512          # 8 query tiles
NEGB = -1e15            # exp(x + NEGB) == 0 in fp32
SCALE = 1.0 / math.sqrt(HD)

_MAX_WAITS = 1


def _split_excess_waits(nc):
    # This walrus build accepts a single sync-wait per instruction; move
    # excess waits onto same-engine NoOps inserted immediately before.
    for bb in nc.m.functions[0].blocks:
        new_insts = []
        changed = False
        for inst in bb.instructions:
            si = inst.sync_info
            if si is not None and si.on_wait and len(si.on_wait) > _MAX_WAITS:
                waits = list(si.on_wait)
                head, tail = waits[:-_MAX_WAITS], waits[-_MAX_WAITS:]
                k = 0
                while head:
                    chunk, head = head[:_MAX_WAITS], head[_MAX_WAITS:]
                    new_insts.append(mybir.InstNoOp(
                        name=f"{inst.name}-ws{k}", engine=inst.engine,
                        ins=[], outs=[],
                        sync_info=mybir.SyncInfo(on_wait=chunk, on_update=[])))
                    k += 1
                inst.sync_info = mybir.SyncInfo(
                    on_wait=tail, on_update=list(si.on_update))
                changed = True
            new_insts.append(inst)
        if changed:
            bb.instructions = new_insts


def _build_program(with_bias):
    nc = bass.Bass("TRN2", target_bir_lowering=False, debug=False)
    xT = nc.dram_tensor("xT", [DIN, S], F32R, kind="ExternalInput").ap()
    wqk = nc.dram_tensor("wqk", [DIN, HL * 128], F32R, kind="ExternalInput").ap()
    wv = nc.dram_tensor("wv", [DIN, HL * 64], F32R, kind="ExternalInput").ap()
    wo = nc.dram_tensor("wo", [HL * 64, EMB], F32R, kind="ExternalInput").ap()
    bqkT = nc.dram_tensor("bqkT", [1, HL * 128], F32R, kind="ExternalInput").ap()
    bv = nc.dram_tensor("bv", [1, HL * 64], F32R, kind="ExternalInput").ap()
    padb = nc.dram_tensor("padb", [128, NKC], F32, kind="ExternalInput").ap()
    mask = nc.dram_tensor("mask", [128, 384], BF16, kind="ExternalInput").ap()
    onesr = nc.dram_tensor("onesr", [1, 512], F32R, kind="ExternalInput").ap()
    y = nc.dram_tensor("y", [S, EMB], F32, kind="ExternalOutput").ap()

    import contextlib
    with tile.TileContext(nc) as tc, contextlib.ExitStack() as ctx:
        cpool = ctx.enter_context(tc.tile_pool(name="const", bufs=1))
        xpool = ctx.enter_context(tc.tile_pool(name="x", bufs=16))
        qkpool = ctx.enter_context(tc.tile_pool(name="qkt", bufs=1))
        vpool = ctx.enter_context(tc.tile_pool(name="v", bufs=1))
        ptpool = ctx.enter_context(tc.tile_pool(name="pt", bufs=40))
        opool = ctx.enter_context(tc.tile_pool(name="outp", bufs=4))
        rpool = ctx.enter_context(tc.tile_pool(name="rec", bufs=2))
        ypool = ctx.enter_context(tc.tile_pool(name="y", bufs=4))
        pp = ctx.enter_context(tc.tile_pool(name="pp", bufs=2, space="PSUM"))
        ypp = ctx.enter_context(tc.tile_pool(name="ypp", bufs=2, space="PSUM"))
        stp = ctx.enter_context(tc.tile_pool(name="st", bufs=2, space="PSUM"))
        pvp = ctx.enter_context(tc.tile_pool(name="pv", bufs=2, space="PSUM"))

        # constants
        wqk_t = []
        for kk in range(8):
            t = cpool.tile([128, HL * 128], F32R, tag=f"wqk{kk}", name=f"wqk{kk}")
            nc.sync.dma_start(t[:], wqk[kk * 128:(kk + 1) * 128, :])
            wqk_t.append(t)

        # phase 1: QKV projection.
        # qT2[p] = q^T for heads 2p (partitions 0:64) and 2p+1 (64:128), bf16
        # kT2[p] = k^T likewise; v[kc] = [128 tok, 4*65] bf16 (+ones col)
        qT2 = [qkpool.tile([128, S], BF16, tag=f"qt{p}", name=f"qt{p}")
               for p in range(2)]
        kT2 = [qkpool.tile([128, S], BF16, tag=f"kt{p}", name=f"kt{p}")
               for p in range(2)]
        v_t = [vpool.tile([128, HL * 65], BF16, tag=f"v{kc}", name=f"v{kc}")
               for kc in range(NKC)]
        def qkv_tile(tt):
            if tt == 0:
                xs = x0
            else:
                xs = []
                for kk in range(8):
                    t = xpool.tile([128, 512], F32R, tag="xt", name=f"xt{tt}_{kk}")
                    nc.sync.dma_start(t[:], xT[kk * 128:(kk + 1) * 128,
                                                tt * 512:(tt + 1) * 512])
                    xs.append(t)
            for qk in range(2):          # 0 -> q of all heads, 1 -> k
                for p2 in range(2):      # head pair
                    c0 = qk * 256 + p2 * 128
                    ps = pp.tile([128, 512], F32, tag="pp", name=f"qk{tt}_{qk}_{p2}")
                    for kk in range(8):
                        nc.tensor.matmul(ps[:], wqk_t[kk][:, c0:c0 + 128],
                                         xs[kk][:], start=(kk == 0),
                                         stop=(kk == 7 and not with_bias))
                    if with_bias:
                        nc.tensor.matmul(ps[:], bqkT_t[:, c0:c0 + 128],
                                         onesr_t[:], start=False, stop=True)
                    dst = qT2[p2] if qk == 0 else kT2[p2]
                    nc.scalar.copy(dst[:, tt * 512:(tt + 1) * 512], ps[:])
            for m in range(4):
                ps = pp.tile([128, HL * 64], F32, tag="pp", name=f"v{tt}_{m}")
                for kk in range(8):
                    nc.tensor.matmul(ps[:], xs[kk][:, m * 128:(m + 1) * 128],
                                     wv_t[kk][:], start=(kk == 0),
                                     stop=(kk == 7 and not with_bias))
                if with_bias:
                    nc.tensor.matmul(ps[:], onesr_t[:, 0:128], bv_t[:],
                                     start=False, stop=True)
                kc = tt * 4 + m
                vv = v_t[kc][:].rearrange("p (h c) -> p h c", h=HL)
                nc.vector.tensor_copy(
                    vv[:, :, 0:64], ps[:].rearrange("p (h c) -> p h c", h=HL))
                nc.gpsimd.memset(vv[:, :, 64:65], 1.0)

        wo_t = []
        for p in range(2):
            t = cpool.tile([128, EMB], F32R, tag=f"wo{p}", name=f"wo{p}")
            nc.sync.dma_start(t[:], wo[p * 128:(p + 1) * 128, :])
            wo_t.append(t)

        # phase 2+3 interleaved per 512-query tile:
        #   pair-packed P^T production (QK -> exp -> band mask on GPSIMD),
        #   PV accumulate out^T, normalize, O-projection + y DMA.
        pts = [{} for _ in range(HL)]

        def make_pt(j, kc):
            qlo = max(0, 128 * kc - 128)
            qhi = min(S, 128 * kc + 256)
            w = qhi - qlo
            p2, hf = j // 2, (j % 2) * 64
            st = stp.tile([128, w], F32, tag="st", name=f"st{j}_{kc}")
            nc.tensor.matmul(st[:],
                             kT2[p2][hf:hf + 64, kc * 128:(kc + 1) * 128],
                             qT2[p2][hf:hf + 64, qlo:qhi],
                             start=True, stop=True)
            pt = ptpool.tile([128, w], BF16, tag="pt", name=f"pt{j}_{kc}")
            nc.scalar.activation(pt[:], st[:], mybir.ActivationFunctionType.Exp,
                                 bias=padb_t[:, kc:kc + 1], scale=SCALE)
            # band mask via affine_select on the edge regions (middle third of
            # the generic 384-wide window is always valid).
            ms = qlo - 128 * kc + 128
            lo_w = max(0, 128 - ms)
            hi_s = max(0, 256 - ms)
            if lo_w > 0:
                # keep iff i - j >= 0 with i = ms + y; expr = ms + y - x
                nc.gpsimd.affine_select(
                    out=pt[:, 0:lo_w], in_=pt[:, 0:lo_w],
                    compare_op=mybir.AluOpType.is_ge, fill=0.0,
                    base=ms, pattern=[[1, lo_w]], channel_multiplier=-1)
            if hi_s < w:
                # keep iff j + 256 - i >= 0 with i = ms + hi_s + y
                nc.gpsimd.affine_select(
                    out=pt[:, hi_s:w], in_=pt[:, hi_s:w],
                    compare_op=mybir.AluOpType.is_ge, fill=0.0,
                    base=256 - ms - hi_s, pattern=[[-1, w - hi_s]],
                    channel_multiplier=1)
            pts[j][kc] = (pt, 0)

        def pv_group(j, qt, outp):
            t0 = qt * 512
            ps = pvp.tile([65, 512], F32, tag="pv", name=f"pv{j}_{qt}")
            nc.vector.memset(ps[:], 0.0)
            ks = [kc for kc in range(4 * qt - 1, 4 * qt + 5) if 0 <= kc < NKC]
            for i, kc in enumerate(ks):
                qlo = max(0, 128 * kc - 128)
                qhi = min(S, 128 * kc + 256)
                a = max(qlo, t0)
                b2 = min(qhi, t0 + 512)
                pt, off = pts[j][kc]
                nc.tensor.matmul(
                    ps[:, a - t0:b2 - t0],
                    v_t[kc][:, j * 65:(j + 1) * 65],
                    pt[:, off + a - qlo:off + b2 - qlo],
                    start=False, stop=(i == len(ks) - 1),
                    skip_group_check=True)
            rec = rpool.tile([1, 512], F32R, tag="rec", name=f"rec{j}_{qt}")
            with nc.allow_low_precision(reason="f32r rounding for matmul"):
                nc.vector.reciprocal(rec[:], ps[64:65, :])
            # copy unnormalized out^T to SBUF in parallel with the reciprocal;
            # the multiply then reads the broadcast reciprocal from PSUM.
            u64 = rpool.tile([64, 512], F32, tag="rec64", name=f"u64{j}_{qt}")
            nc.scalar.copy(u64[:], ps[0:64, :])
            bc = ypp.tile([64, 512], F32, tag="ypp", name=f"bc{j}_{qt}")
            nc.tensor.matmul(bc[:], onesr_t[:, 0:64], rec[:], start=True, stop=True)
            p, half = j // 2, j % 2
            with nc.allow_low_precision(reason="f32r rounding for matmul"):
                nc.vector.tensor_mul(
                    outp[p][half * 64:half * 64 + 64, :], u64[:], bc[:])

        def attn_tile(qt):
            lo = 4 * qt + 1 if qt > 0 else 0
            hi = min(4 * qt + 5, NKC)
            for kc in range(lo, hi):
                for j in range(HL):
                    make_pt(j, kc)
            outp = [opool.tile([128, 512], F32R, tag=f"op{p}", name=f"op{p}_{qt}")
                    for p in range(2)]
            for j in range(HL):
                pv_group(j, qt, outp)
            for m in range(4):
                for n in range(2):
                    ypool_ps = pp if qt >= 6 else ypp
                    ps = ypool_ps.tile([128, 512], F32,
                                       tag="pp" if qt >= 6 else "ypp",
                                       name=f"y{qt}_{m}_{n}")
                    for p in range(2):
                        nc.tensor.matmul(ps[:], outp[p][:, m * 128:(m + 1) * 128],
                                         wo_t[p][:, n * 512:(n + 1) * 512],
                                         start=(p == 0), stop=(p == 1))
                    yt = ypool.tile([128, 512], F32, tag="y",
                                    name=f"yt{qt}_{m}_{n}")
                    if (m + n) % 2 == 0:
                        nc.vector.tensor_copy(yt[:], ps[:])
                    else:
                        nc.scalar.copy(yt[:], ps[:])
                    nc.sync.dma_start(
                        y[qt * 512 + m * 128:qt * 512 + (m + 1) * 128,
                          n * 512:(n + 1) * 512], yt[:])

        x0 = []
        for kk in range(8):
            t = xpool.tile([128, 512], F32R, tag="xt", name=f"xt0_{kk}")
            nc.sync.dma_start(t[:], xT[kk * 128:(kk + 1) * 128, 0:512])
            x0.append(t)
        wv_t = []
        for kk in range(8):
            t = cpool.tile([128, HL * 64], F32R, tag=f"wv{kk}", name=f"wv{kk}")
            nc.sync.dma_start(t[:], wv[kk * 128:(kk + 1) * 128, :])
            wv_t.append(t)
        bqkT_t = cpool.tile([1, HL * 128], F32R, tag="bqkT")
        nc.sync.dma_start(bqkT_t[:], bqkT[:])
        bv_t = cpool.tile([1, HL * 64], F32R, tag="bv")
        nc.sync.dma_start(bv_t[:], bv[:])
        padb_t = cpool.tile([128, NKC], F32, tag="padb")
        nc.sync.dma_start(padb_t[:], padb[:])
        mask_t = cpool.tile([128, 384], BF16, tag="mask")
        nc.sync.dma_start(mask_t[:], mask[:])
        onesr_t = cpool.tile([1, 512], F32R, tag="onesr")
        nc.sync.dma_start(onesr_t[:], onesr[:])
        onesb_t = cpool.tile([1, 65], BF16, tag="onesb")
        nc.vector.memset(onesb_t[:], 1.0)
        zrow_t = cpool.tile([1, 512], BF16, tag="zrow")
        nc.vector.memset(zrow_t[:], 0.0)
        for tt in range(8):
            qkv_tile(tt)
            if tt >= 1:
                attn_tile(tt - 1)
        attn_tile(NQT - 1)

    _split_excess_waits(nc)
    return nc


_NC = {}


def _get_nc(with_bias=False):
    if with_bias not in _NC:
        _NC[with_bias] = _build_program(with_bias)
    return _NC[with_bias]


def kernel(x, padding_mask, W_qkv, b_qkv, W_o, b_o):
    x = np.asarray(x, np.float32)
    padding_mask = np.asarray(padding_mask)
    W_qkv = np.asarray(W_qkv, np.float32)
    b_qkv = np.asarray(b_qkv, np.float32)
    W_o = np.asarray(W_o, np.float32)
    b_o = np.asarray(b_o, np.float32)

    # band mask (transposed layout): key j_local (partition), query col i
    # covering queries [128*kc - 128, 128*kc + 256); valid iff j <= i <= j+256
    jj = np.arange(128)[:, None]
    ii = np.arange(384)[None, :]
    mask01 = ((ii >= jj) & (ii <= jj + 2 * W)).astype(BF16NP)
    ones_r = np.ones((1, 512), np.float32)

    in_maps = []
    for c in range(8):
        b = c // 4
        g = c % 4
        heads = [g * HL + j for j in range(HL)]
        xT = np.ascontiguousarray(x[b].T)
        wqk = np.empty((DIN, HL * 128), np.float32)
        wv = np.empty((DIN, HL * 64), np.float32)
        wo = np.empty((HL * 64, EMB), np.float32)
        bqkT = np.empty((1, HL * 128), np.float32)
        bv = np.empty((1, HL * 64), np.float32)
        for j, h in enumerate(heads):
            wqk[:, j * 64:(j + 1) * 64] = W_qkv[:, h * 192:h * 192 + 64]
            wqk[:, 256 + j * 64:256 + (j + 1) * 64] = \
                W_qkv[:, h * 192 + 64:h * 192 + 128]
            wv[:, j * 64:(j + 1) * 64] = W_qkv[:, h * 192 + 128:h * 192 + 192]
            wo[j * 64:(j + 1) * 64, :] = W_o[h * 64:(h + 1) * 64, :]
            bqkT[0, j * 64:(j + 1) * 64] = b_qkv[h * 192:h * 192 + 64]
            bqkT[0, 256 + j * 64:256 + (j + 1) * 64] = \
                b_qkv[h * 192 + 64:h * 192 + 128]
            bv[0, j * 64:(j + 1) * 64] = b_qkv[h * 192 + 128:h * 192 + 192]
        padb = np.where(padding_mask[b].reshape(NKC, 128).T.astype(bool),
                        0.0, NEGB).astype(np.float32)
        in_maps.append({
            "xT": xT, "wqk": wqk, "wv": wv, "wo": wo, "bqkT": bqkT, "bv": bv,
            "padb": np.ascontiguousarray(padb), "mask": mask01, "onesr": ones_r,
        })

    with_bias = bool(np.any(b_qkv != 0))
    nc = _get_nc(with_bias)
    res = bass_utils.run_bass_kernel_spmd(nc, in_maps, core_ids=list(range(8)))
    out = np.zeros((B, S, EMB), np.float32)
    for c in range(8):
        out[c // 4] += res.results[c]["y"]
    out += b_o
    return out


# revision 23
# speedup vs baseline: 1.4515x; 1.0331x over previous
# Banded (sliding-window) multi-head attention for Trainium2, 8 NeuronCores.
# Sharding: batch x head-group (2 batches x 4 groups of 4 heads). QKV is
# column-parallel, W_o row-parallel (megatron TP); host sums the 4 partial
# outputs per batch and adds b_o.
import math
import numpy as np
import ml_dtypes

import concourse.bass as bass
import concourse.tile as tile
import concourse.mybir as mybir
import concourse.bass_utils as bass_utils

F32 = mybir.dt.float32
F32R = mybir.dt.float32r
BF16 = mybir.dt.bfloat16
BF16NP = ml_dtypes.bfloat16

B, S, DIN, EMB, NH, WIN = 2, 4096, 1024, 1024, 16, 256
HD = EMB // NH          # 64
W = WIN // 2            # 128 one-sided window
HL = 4                  # heads per core
NKC = S // 128          # 32 key chunks
NQT = S // 512          # 8 query tiles
NEGB = -1e15            # exp(x + NEGB) == 0 in fp32
SCALE = 1.0 / math.sqrt(HD)

_MAX_WAITS = 1


def _split_excess_waits(nc):
    # This walrus build accepts a single sync-wait per instruction; move
    # excess waits onto same-engine NoOps inserted immediately before.
    for bb in nc.m.functions[0].blocks:
        new_insts = []
        changed = False
        for inst in bb.instructions:
            si = inst.sync_info
            if si is not None and si.on_wait and len(si.on_wait) > _MAX_WAITS:
                waits = list(si.on_wait)
                head, tail = waits[:-_MAX_WAITS], waits[-_MAX_WAITS:]
                k = 0
                while head:
                    chunk, head = head[:_MAX_WAITS], head[_MAX_WAITS:]
                    new_insts.append(mybir.InstNoOp(
                        name=f"{inst.name}-ws{k}", engine=inst.engine,
                        ins=[], outs=[],
                        sync_info=mybir.SyncInfo(on_wait=chunk, on_update=[])))
                    k += 1
                inst.sync_info = mybir.SyncInfo(
                    on_wait=tail, on_update=list(si.on_update))
                changed = True
            new_insts.append(inst)
        if changed:
            bb.instructions = new_insts


def _build_program(with_bias):
    nc = bass.Bass("TRN2", target_bir_lowering=False, debug=False)
    xT = nc.dram_tensor("xT", [DIN, S], F32R, kind="ExternalInput").ap()
    wqk = nc.dram_tensor("wqk", [DIN, HL * 128], F32R, kind="ExternalInput").ap()
    wv = nc.dram_tensor("wv", [DIN, HL * 64], F32R, kind="ExternalInput").ap()
    wo = nc.dram_tensor("wo", [HL * 64, EMB], F32R, kind="ExternalInput").ap()
    bqkT = nc.dram_tensor("bqkT", [1, HL * 128], F32R, kind="ExternalInput").ap()
    bv = nc.dram_tensor("bv", [1, HL * 64], F32R, kind="ExternalInput").ap()
    padb = nc.dram_tensor("padb", [128, NKC], F32, kind="ExternalInput").ap()
    mask = nc.dram_tensor("mask", [128, 384], BF16, kind="ExternalInput").ap()
    onesr = nc.dram_tensor("onesr", [1, 512], F32R, kind="ExternalInput").ap()
    y = nc.dram_tensor("y", [S, EMB], F32, kind="ExternalOutput").ap()

    import contextlib
    with tile.TileContext(nc) as tc, contextlib.ExitStack() as ctx:
        cpool = ctx.enter_context(tc.tile_pool(name="const", bufs=1))
        xpool = ctx.enter_context(tc.tile_pool(name="x", bufs=20))
        qkpool = ctx.enter_context(tc.tile_pool(name="qkt", bufs=1))
        vpool = ctx.enter_context(tc.tile_pool(name="v", bufs=1))
        ptpool = ctx.enter_context(tc.tile_pool(name="pt", bufs=40))
        opool = ctx.enter_context(tc.tile_pool(name="outp", bufs=4))
        rpool = ctx.enter_context(tc.tile_pool(name="rec", bufs=3))
        ypool = ctx.enter_context(tc.tile_pool(name="y", bufs=6))
        pp = ctx.enter_context(tc.tile_pool(name="pp", bufs=2, space="PSUM"))
        ypp = ctx.enter_context(tc.tile_pool(name="ypp", bufs=2, space="PSUM"))
        stp = ctx.enter_context(tc.tile_pool(name="st", bufs=2, space="PSUM"))
        pvp = ctx.enter_context(tc.tile_pool(name="pv", bufs=2, space="PSUM"))

        # constants: interleave wqk chunks with the first x slab so the
        # first QKV matmuls can start as soon as possible
        wqk_t = []
        x0 = []
        for kk in range(8):
            t = cpool.tile([128, HL * 128], F32R, tag=f"wqk{kk}", name=f"wqk{kk}")
            nc.sync.dma_start(t[:], wqk[kk * 128:(kk + 1) * 128, :])
            wqk_t.append(t)
            t2 = xpool.tile([128, 512], F32R, tag="xt", name=f"xt0_{kk}")
            nc.sync.dma_start(t2[:], xT[kk * 128:(kk + 1) * 128, 0:512])
            x0.append(t2)

        # phase 1: QKV projection.
        # qT2[p] = q^T for heads 2p (partitions 0:64) and 2p+1 (64:128), bf16
        # kT2[p] = k^T likewise; v[kc] = [128 tok, 4*65] bf16 (+ones col)
        qT2 = [qkpool.tile([128, S], BF16, tag=f"qt{p}", name=f"qt{p}")
               for p in range(2)]
        kT2 = [qkpool.tile([128, S], BF16, tag=f"kt{p}", name=f"kt{p}")
               for p in range(2)]
        v_t = [vpool.tile([128, HL * 65], BF16, tag=f"v{kc}", name=f"v{kc}")
               for kc in range(NKC)]
        def qkv_tile(tt):
            if tt == 0:
                xs = x0
            else:
                xs = []
                for kk in range(8):
                    t = xpool.tile([128, 512], F32R, tag="xt", name=f"xt{tt}_{kk}")
                    nc.sync.dma_start(t[:], xT[kk * 128:(kk + 1) * 128,
                                                tt * 512:(tt + 1) * 512])
                    xs.append(t)
            for qk in range(2):          # 0 -> q of all heads, 1 -> k
                for p2 in range(2):      # head pair
                    c0 = qk * 256 + p2 * 128
                    ps = pp.tile([128, 512], F32, tag="pp", name=f"qk{tt}_{qk}_{p2}")
                    for kk in range(8):
                        nc.tensor.matmul(ps[:], wqk_t[kk][:, c0:c0 + 128],
                                         xs[kk][:], start=(kk == 0),
                                         stop=(kk == 7 and not with_bias))
                    if with_bias:
                        nc.tensor.matmul(ps[:], bqkT_t[:, c0:c0 + 128],
                                         onesr_t[:], start=False, stop=True)
                    dst = qT2[p2] if qk == 0 else kT2[p2]
                    nc.scalar.copy(dst[:, tt * 512:(tt + 1) * 512], ps[:])
            for m in range(4):
                ps = pp.tile([128, HL * 64], F32, tag="pp", name=f"v{tt}_{m}")
                for kk in range(8):
                    nc.tensor.matmul(ps[:], xs[kk][:, m * 128:(m + 1) * 128],
                                     wv_t[kk][:], start=(kk == 0),
                                     stop=(kk == 7 and not with_bias))
                if with_bias:
                    nc.tensor.matmul(ps[:], onesr_t[:, 0:128], bv_t[:],
                                     start=False, stop=True)
                kc = tt * 4 + m
                vv = v_t[kc][:].rearrange("p (h c) -> p h c", h=HL)
                nc.vector.tensor_copy(
                    vv[:, :, 0:64], ps[:].rearrange("p (h c) -> p h c", h=HL))
                nc.gpsimd.memset(vv[:, :, 64:65], 1.0)

        wo_t = []
        for p in range(2):
            t = cpool.tile([128, EMB], F32R, tag=f"wo{p}", name=f"wo{p}")
            nc.sync.dma_start(t[:], wo[p * 128:(p + 1) * 128, :])
            wo_t.append(t)

        # phase 2+3 interleaved per 512-query tile:
        #   pair-packed P^T production (QK -> exp -> band mask on GPSIMD),
        #   PV accumulate out^T, normalize, O-projection + y DMA.
        pts = [{} for _ in range(HL)]

        def make_pt(j, kc):
            qlo = max(0, 128 * kc - 128)
            qhi = min(S, 128 * kc + 256)
            w = qhi - qlo
            p2, hf = j // 2, (j % 2) * 64
            st = stp.tile([128, w], F32, tag="st", name=f"st{j}_{kc}")
            nc.tensor.matmul(st[:],
                             kT2[p2][hf:hf + 64, kc * 128:(kc + 1) * 128],
                             qT2[p2][hf:hf + 64, qlo:qhi],
                             start=True, stop=True)
            pt = ptpool.tile([128, w], BF16, tag="pt", name=f"pt{j}_{kc}")
            nc.scalar.activation(pt[:], st[:], mybir.ActivationFunctionType.Exp,
                                 bias=padb_t[:, kc:kc + 1], scale=SCALE)
            # band mask via affine_select on the edge regions (middle third of
            # the generic 384-wide window is always valid).
            ms = qlo - 128 * kc + 128
            lo_w = max(0, 128 - ms)
            hi_s = max(0, 256 - ms)
            if lo_w > 0:
                # keep iff i - j >= 0 with i = ms + y; expr = ms + y - x
                nc.gpsimd.affine_select(
                    out=pt[:, 0:lo_w], in_=pt[:, 0:lo_w],
                    compare_op=mybir.AluOpType.is_ge, fill=0.0,
                    base=ms, pattern=[[1, lo_w]], channel_multiplier=-1)
            if hi_s < w:
                # keep iff j + 256 - i >= 0 with i = ms + hi_s + y
                nc.gpsimd.affine_select(
                    out=pt[:, hi_s:w], in_=pt[:, hi_s:w],
                    compare_op=mybir.AluOpType.is_ge, fill=0.0,
                    base=256 - ms - hi_s, pattern=[[-1, w - hi_s]],
                    channel_multiplier=1)
            pts[j][kc] = (pt, 0)

        def pv_group(j, qt, outp):
            t0 = qt * 512
            ps = pvp.tile([65, 512], F32, tag="pv", name=f"pv{j}_{qt}")
            nc.vector.memset(ps[:], 0.0)
            ks = [kc for kc in range(4 * qt - 1, 4 * qt + 5) if 0 <= kc < NKC]
            for i, kc in enumerate(ks):
                qlo = max(0, 128 * kc - 128)
                qhi = min(S, 128 * kc + 256)
                a = max(qlo, t0)
                b2 = min(qhi, t0 + 512)
                pt, off = pts[j][kc]
                nc.tensor.matmul(
                    ps[:, a - t0:b2 - t0],
                    v_t[kc][:, j * 65:(j + 1) * 65],
                    pt[:, off + a - qlo:off + b2 - qlo],
                    start=False, stop=(i == len(ks) - 1),
                    skip_group_check=True)
            rec = rpool.tile([1, 512], F32R, tag="rec", name=f"rec{j}_{qt}")
            with nc.allow_low_precision(reason="f32r rounding for matmul"):
                nc.vector.reciprocal(rec[:], ps[64:65, :])
            # copy unnormalized out^T to SBUF in parallel with the reciprocal;
            # the multiply then reads the broadcast reciprocal from PSUM.
            u64 = rpool.tile([64, 512], F32, tag="rec64", name=f"u64{j}_{qt}")
            nc.scalar.copy(u64[:], ps[0:64, :])
            bc = ypp.tile([64, 512], F32, tag="ypp", name=f"bc{j}_{qt}")
            nc.tensor.matmul(bc[:], onesr_t[:, 0:64], rec[:], start=True, stop=True)
            p, half = j // 2, j % 2
            with nc.allow_low_precision(reason="f32r rounding for matmul"):
                nc.vector.tensor_mul(
                    outp[p][half * 64:half * 64 + 64, :], u64[:], bc[:])

        def attn_tile(qt):
            lo = 4 * qt + 1 if qt > 0 else 0
            hi = min(4 * qt + 5, NKC)
            for kc in range(lo, hi):
                for j in range(HL):
                    make_pt(j, kc)
            outp = [opool.tile([128, 512], F32R, tag=f"op{p}", name=f"op{p}_{qt}")
                    for p in range(2)]
            for j in range(HL):
                pv_group(j, qt, outp)
            for m in range(4):
                for n in range(2):
                    ps = ypp.tile([128, 512], F32, tag="ypp",
                                  name=f"y{qt}_{m}_{n}")
                    for p in range(2):
                        nc.tensor.matmul(ps[:], outp[p][:, m * 128:(m + 1) * 128],
                                         wo_t[p][:, n * 512:(n + 1) * 512],
                                         start=(p == 0), stop=(p == 1))
                    yt = ypool.tile([128, 512], F32, tag="y",
                                    name=f"yt{qt}_{m}_{n}")
                    if (m + n) % 2 == 0:
                        nc.vector.tensor_copy(yt[:], ps[:])
                    else:
                        nc.scalar.copy(yt[:], ps[:])
                    nc.sync.dma_start(
                        y[qt * 512 + m * 128:qt * 512 + (m + 1) * 128,
                          n * 512:(n + 1) * 512], yt[:])

        wv_t = []
        for kk in range(8):
            t = cpool.tile([128, HL * 64], F32R, tag=f"wv{kk}", name=f"wv{kk}")
            nc.sync.dma_start(t[:], wv[kk * 128:(kk + 1) * 128, :])
            wv_t.append(t)
        bqkT_t = cpool.tile([1, HL * 128], F32R, tag="bqkT")
        nc.sync.dma_start(bqkT_t[:], bqkT[:])
        bv_t = cpool.tile([1, HL * 64], F32R, tag="bv")
        nc.sync.dma_start(bv_t[:], bv[:])
        padb_t = cpool.tile([128, NKC], F32, tag="padb")
        nc.sync.dma_start(padb_t[:], padb[:])
        mask_t = cpool.tile([128, 384], BF16, tag="mask")
        nc.sync.dma_start(mask_t[:], mask[:])
        onesr_t = cpool.tile([1, 512], F32R, tag="onesr")
        nc.sync.dma_start(onesr_t[:], onesr[:])
        onesb_t = cpool.tile([1, 65], BF16, tag="onesb")
        nc.vector.memset(onesb_t[:], 1.0)
        zrow_t = cpool.tile([1, 512], BF16, tag="zrow")
        nc.vector.memset(zrow_t[:], 0.0)
        for tt in range(8):
            qkv_tile(tt)
            if tt >= 1:
                attn_tile(tt - 1)
        attn_tile(NQT - 1)

    _split_excess_waits(nc)
    return nc


_NC = {}


def _get_nc(with_bias=False):
    if with_bias not in _NC:
        _NC[with_bias] = _build_program(with_bias)
    return _NC[with_bias]


def kernel(x, padding_mask, W_qkv, b_qkv, W_o, b_o):
    x = np.asarray(x, np.float32)
    padding_mask = np.asarray(padding_mask)
    W_qkv = np.asarray(W_qkv, np.float32)
    b_qkv = np.asarray(b_qkv, np.float32)
    W_o = np.asarray(W_o, np.float32)
    b_o = np.asarray(b_o, np.float32)

    # band mask (transposed layout): key j_local (partition), query col i
    # covering queries [128*kc - 128, 128*kc + 256); valid iff j <= i <= j+256
    jj = np.arange(128)[:, None]
    ii = np.arange(384)[None, :]
    mask01 = ((ii >= jj) & (ii <= jj + 2 * W)).astype(BF16NP)
    ones_r = np.ones((1, 512), np.float32)

    in_maps = []
    for c in range(8):
        b = c // 4
        g = c % 4
        heads = [g * HL + j for j in range(HL)]
        xT = np.ascontiguousarray(x[b].T)
        wqk = np.empty((DIN, HL * 128), np.float32)
        wv = np.empty((DIN, HL * 64), np.float32)
        wo = np.empty((HL * 64, EMB), np.float32)
        bqkT = np.empty((1, HL * 128), np.float32)
        bv = np.empty((1, HL * 64), np.float32)
        for j, h in enumerate(heads):
            wqk[:, j * 64:(j + 1) * 64] = W_qkv[:, h * 192:h * 192 + 64]
            wqk[:, 256 + j * 64:256 + (j + 1) * 64] = \
                W_qkv[:, h * 192 + 64:h * 192 + 128]
            wv[:, j * 64:(j + 1) * 64] = W_qkv[:, h * 192 + 128:h * 192 + 192]
            wo[j * 64:(j + 1) * 64, :] = W_o[h * 64:(h + 1) * 64, :]
            bqkT[0, j * 64:(j + 1) * 64] = b_qkv[h * 192:h * 192 + 64]
            bqkT[0, 256 + j * 64:256 + (j + 1) * 64] = \
                b_qkv[h * 192 + 64:h * 192 + 128]
            bv[0, j * 64:(j + 1) * 64] = b_qkv[h * 192 + 128:h * 192 + 192]
        padb = np.where(padding_mask[b].reshape(NKC, 128).T.astype(bool),
                        0.0, NEGB).astype(np.float32)
        in_maps.append({
            "xT": xT, "wqk": wqk, "wv": wv, "wo": wo, "bqkT": bqkT, "bv": bv,
            "padb": np.ascontiguousarray(padb), "mask": mask01, "onesr": ones_r,
        })

    with_bias = bool(np.any(b_qkv != 0))
    nc = _get_nc(with_bias)
    res = bass_utils.run_bass_kernel_spmd(nc, in_maps, core_ids=list(range(8)))
    out = np.zeros((B, S, EMB), np.float32)
    for c in range(8):
        out[c // 4] += res.results[c]["y"]
    out += b_o
    return out
